# revision 20
# baseline (speedup 1.0000x reference)
"""Group MoE layer (2 groups x 4 experts, top-1 group / top-2 expert routing)
on 8 Trainium2 NeuronCores via expert parallelism.

Strategy:
  - Host computes the (tiny) routing: language-gate argmax over groups,
    per-group expert top-k + softmax weights.
  - Tokens are dispatched by (group, expert) assignment: core c = g*4+e
    receives exactly the tokens routed to expert (g, e), padded to a common
    capacity C (SPMD: all cores run the same program).
  - Each core runs the dense FFN for its expert:
        Y^T = W2 @ relu(W1 @ X^T + b1) + b2      (tokens in the moving dim)
    with bf16 weights/activations and fp32 PSUM accumulation.
  - The capacity remainder above a multiple of 512 (e.g. C=1051 -> 2x512+27)
    does NOT get its own pass over the weights: a separate pass costs a full
    LDWEIGHTS sweep (512 weight tiles x ~56ns ~= 29us) regardless of token
    count. Instead the remainder rides block 0's weight stream: each weight
    tile issues a second matmul (N=rem) reusing the stationary operand,
    costing ~25ns extra per tile (~13us total instead of 29us).

HW behaviors this schedule is built around (all measured via ntff traces):
  - DMA transfers whose per-partition rows are < 8KB crawl (~34GB/s) while
    any engine is busy, so every tensor is packed with >= 8KB rows: the rem
    x-columns are interleaved into x block 0, and both biases live in one
    zero-padded [128, 2080] f32 tensor.
  - The scalar engine executes the activations AND its DMA triggers in one
    FIFO; a trigger blocks once the HWDGE ring saturates (~4+ queued), which
    deadlocks ACT -> PSUM-free -> matmul. So the scalar ring carries exactly
    2 triggers; the whole W1 -> W2 -> x stream serializes on the sync ring
    in consumption order.
  - The compile-time Tile scheduler prices DMA optimistically and coalesces
    semaphore thresholds, so mm2's accumulation runs hi high->low: its first
    matmul then depends on the LAST h1 tile and can't be hoisted (with a
    not-yet-landed W2 wait) into mm1's stream.
  - The PE's HAM clock gate holds 1.2GHz until ~3.4us of sustained activity:
    ~36 throwaway matmuls on a zeroed scratch tile warm it up during the
    initial loads, so the real stream starts at 2.4GHz.
"""

import numpy as np
import ml_dtypes

import concourse.bacc as bacc
import concourse.mybir as mybir
from concourse import tile
from concourse import bass_utils

B, L, D, H = 2, 2048, 1024, 4096
G, E = 2, 4
NCORES = G * E
PART = 128
TOK_BLK = 512
W2GRP = 4                       # h-tiles per merged W2 tile (8KB rows, 1MB)
BPAD = 2080                     # bias tensor f32 cols (8KB+ rows: no crawl)
# W1 chunk widths (columns of H per DMA); the first is smallest so the PE
# start only gates on 1MB of W1.
W1CHUNKS = (512, 512, 1024, 1024, 1024)
assert sum(W1CHUNKS) == H

_BF16 = ml_dtypes.bfloat16

_program_cache: dict[tuple, object] = {}


def _build(nfull: int, rem: int, rem_pad: int, d: int = D, h: int = H):
    """Per-core expert FFN program: nfull token blocks of 512 plus an
    optional remainder of `rem` tokens merged into block 0's weight stream
    (the rem columns are interleaved into block 0's x layout)."""
    key = (nfull, rem, rem_pad, d, h)
    if key in _program_cache:
        return _program_cache[key]

    nd = d // PART
    nh = h // PART
    ng2 = nh // W2GRP
    span0 = TOK_BLK + rem_pad           # block 0 carries the rem columns
    chunk_of = []
    for c, w in enumerate(W1CHUNKS):
        for off in range(w // PART):
            chunk_of.append((c, off))
    assert len(chunk_of) == nh

    bf16 = mybir.dt.bfloat16
    f32 = mybir.dt.float32

    nc = bacc.Bacc("TRN2", target_bir_lowering=False, debug=False,
                   num_devices=NCORES)

    xt0 = nc.dram_tensor("xt0", [PART, nd * span0], bf16,
                         kind="ExternalInput")
    if nfull > 1:
        xt = nc.dram_tensor("xt", [nfull - 1, PART, nd * TOK_BLK], bf16,
                            kind="ExternalInput")
    w1c = [nc.dram_tensor(f"w1c{c}", [PART, nd * w], bf16,
                          kind="ExternalInput")
           for c, w in enumerate(W1CHUNKS)]
    w2t = nc.dram_tensor("w2t", [ng2, PART, W2GRP * d], bf16,
                         kind="ExternalInput")
    bt = nc.dram_tensor("bt", [PART, BPAD], f32, kind="ExternalInput")
    yt = nc.dram_tensor("yt", [nfull, PART, nd * TOK_BLK], f32,
                        kind="ExternalOutput")
    if rem:
        yr = nc.dram_tensor("yr", [PART, nd * rem_pad], f32,
                            kind="ExternalOutput")

    with tile.TileContext(nc) as tc:
        with (
            tc.tile_pool(name="wpool", bufs=1) as wpool,
            tc.tile_pool(name="h1pool", bufs=nh) as h1pool,
            tc.tile_pool(name="ypool", bufs=1) as ypool,
            tc.tile_pool(name="ps1", bufs=2, space="PSUM") as ps1,
            tc.tile_pool(name="ps2", bufs=2, space="PSUM") as ps2,
        ):
            # PE warm-up (HAM clock gate, see module docstring)
            warm_x = wpool.tile([PART, TOK_BLK], bf16, tag="warm")
            nc.gpsimd.memset(warm_x[:, :], 0.0)
            warm_ps = ps1.tile([PART, TOK_BLK], f32, tag="m")
            for _ in range(36):
                nc.tensor.matmul(warm_ps[:, :], warm_x[:, :PART],
                                 warm_x[:, :], start=True, stop=True)

            w1_sb = [None] * len(W1CHUNKS)

            def load_w1(c, eng):
                t = wpool.tile([PART, nd * W1CHUNKS[c]], bf16, tag=f"w1_{c}")
                eng.dma_start(out=t[:, :], in_=w1c[c].ap()[:, :])
                w1_sb[c] = t

            # scalar ring: exactly 2 fat triggers (biases, then x block 0)
            b_sb = wpool.tile([PART, BPAD], f32, tag="bt")
            nc.scalar.dma_start(out=b_sb[:, :], in_=bt.ap()[:, :])
            x0_sb = wpool.tile([PART, nd * span0], bf16, tag="x_0")
            nc.scalar.dma_start(out=x0_sb[:, :], in_=xt0.ap()[:, :])

            # sync ring: W1 chunks in consumption order, W2, later x blocks
            x_sb = [None] * nfull
            x_sb[0] = x0_sb
            for c in range(len(W1CHUNKS)):
                load_w1(c, nc.sync)
            w2_sb = [None] * ng2
            for gi in range(ng2):
                t = wpool.tile([PART, W2GRP * d], bf16, tag=f"w2_{gi}")
                nc.sync.dma_start(out=t[:, :], in_=w2t.ap()[gi])
                w2_sb[gi] = t
            for blk in range(1, nfull):
                t = wpool.tile([PART, nd * TOK_BLK], bf16, tag=f"x_{blk}")
                nc.sync.dma_start(out=t[:, :], in_=xt.ap()[blk - 1])
                x_sb[blk] = t

            # --- compute passes ------------------------------------------
            for p in range(nfull):
                merged = (p == 0 and rem > 0)
                span = span0 if p == 0 else TOK_BLK
                h1m_tiles = []
                h1r_tiles = []
                for hi in range(nh):
                    c, off = chunk_of[hi]
                    wch = W1CHUNKS[c]
                    psm = ps1.tile([PART, TOK_BLK], f32, tag="m")
                    if merged:
                        psr = ps1.tile([PART, TOK_BLK], f32, tag="r")
                    for di in range(nd):
                        w_ap = w1_sb[c][:, di * wch + off * PART:
                                        di * wch + (off + 1) * PART]
                        nc.tensor.matmul(
                            psm[:, :], w_ap,
                            x_sb[p][:, di * span:di * span + TOK_BLK],
                            start=(di == 0), stop=(di == nd - 1),
                        )
                        if merged:
                            # second matmul on the same stationary weights
                            nc.tensor.matmul(
                                psr[:, :rem], w_ap,
                                x_sb[0][:, di * span + TOK_BLK:
                                        di * span + TOK_BLK + rem],
                                start=(di == 0), stop=(di == nd - 1),
                            )
                    h1m = h1pool.tile([PART, TOK_BLK], bf16, tag="h1m")
                    nc.scalar.activation(h1m[:, :], psm[:, :],
                                         mybir.ActivationFunctionType.Relu,
                                         bias=b_sb[:, hi:hi + 1], scale=1.0)
                    h1m_tiles.append(h1m)
                    if merged:
                        h1r = h1pool.tile([PART, rem_pad], bf16, tag="h1r")
                        nc.scalar.activation(
                            h1r[:, :rem], psr[:, :rem],
                            mybir.ActivationFunctionType.Relu,
                            bias=b_sb[:, hi:hi + 1], scale=1.0)
                        h1r_tiles.append(h1r)

                y = ypool.tile([PART, nd * TOK_BLK], f32, tag="y")
                if merged:
                    y_r = ypool.tile([PART, nd * rem_pad], f32, tag="yr")
                for di in range(nd):
                    ps2m = ps2.tile([PART, TOK_BLK], f32, tag="m")
                    if merged:
                        ps2r = ps2.tile([PART, TOK_BLK], f32, tag="r")
                    # hi runs high->low: the chain's first matmul then needs
                    # the LAST h1 tile, so the compile-time scheduler cannot
                    # hoist mm2 matmuls (whose W2 may still be in flight)
                    # ahead of ready mm1 work in the in-order PE queue.
                    for hi in range(nh - 1, -1, -1):
                        gi, hj = divmod(hi, W2GRP)
                        w_ap = w2_sb[gi][:, hj * d + di * PART:
                                         hj * d + (di + 1) * PART]
                        nc.tensor.matmul(
                            ps2m[:, :], w_ap, h1m_tiles[hi][:, :],
                            start=(hi == nh - 1), stop=(hi == 0),
                        )
                        if merged:
                            nc.tensor.matmul(
                                ps2r[:, :rem], w_ap, h1r_tiles[hi][:, :rem],
                                start=(hi == nh - 1), stop=(hi == 0),
                            )
                    nc.vector.tensor_scalar_add(
                        y[:, di * TOK_BLK:(di + 1) * TOK_BLK], ps2m[:, :],
                        b_sb[:, nh + di:nh + di + 1])
                    # drain several d-tiles per DMA (8KB rows go at line
                    # rate), but keep the final drain a single d-tile so the
                    # post-last-matmul tail transfer is small
                    if di in (3, nd - 2, nd - 1):
                        lo = 0 if di == 3 else (4 if di == nd - 2 else nd - 1)
                        nc.sync.dma_start(
                            out=yt.ap()[p][:, lo * TOK_BLK:(di + 1) * TOK_BLK],
                            in_=y[:, lo * TOK_BLK:(di + 1) * TOK_BLK])
                    if merged:
                        nc.vector.tensor_scalar_add(
                            y_r[:, di * rem_pad:di * rem_pad + rem],
                            ps2r[:, :rem], b_sb[:, nh + di:nh + di + 1])
                if merged:
                    # one drain for the whole remainder block (mid-kernel,
                    # fully overlapped with the rest of the compute)
                    nc.sync.dma_start(out=yr.ap()[:, :], in_=y_r[:, :])

    nc.compile()
    _program_cache[key] = nc
    return nc


def _route(x, bn, Wlg, blg, Wg, k):
    """Numpy replica of the reference routing. Returns per-(g,e) assignment."""
    glog = bn @ Wlg.T + blg                       # (N, G)
    sel_group = np.argmax(glog, axis=1)           # (N,)
    assign = []
    for g in range(Wg.shape[0]):
        logits = x @ Wg[g].T                      # (N, E)
        order = np.argsort(-logits, axis=1, kind="stable")
        sel = order[:, :k]                        # (N, k)
        top = np.take_along_axis(logits, sel, axis=1).astype(np.float32)
        m = top.max(axis=1, keepdims=True)
        ex = np.exp(top - m)
        w = ex / ex.sum(axis=1, keepdims=True)    # (N, k)
        assign.append((sel, w))
    return sel_group, assign


def _pack_x(X, d, nblk, tok_blk):
    """(nblk*tok_blk, d) fp32 -> [nblk, 128, nd*tok_blk] bf16 merged tiles."""
    nd = d // PART
    xt = X.T.astype(_BF16)                        # (d, nblk*tok_blk)
    return np.ascontiguousarray(
        xt.reshape(nd, PART, nblk, tok_blk).transpose(2, 1, 0, 3)
          .reshape(nblk, PART, nd * tok_blk))


def _pack_w1_chunk(W1e_T, d, h0, w):
    """W1e.T slice (d, h0:h0+w) fp32 -> [128, nd*w] bf16."""
    nd = d // PART
    wsl = W1e_T[:, h0:h0 + w].astype(_BF16)       # (d, w)
    return np.ascontiguousarray(
        wsl.reshape(nd, PART, w).transpose(1, 0, 2).reshape(PART, nd * w))


def _pack_w2(W2e, d, h):
    ng2 = h // PART // W2GRP
    w = W2e.T.astype(_BF16)                       # (h, d)
    return np.ascontiguousarray(
        w.reshape(ng2, W2GRP, PART, d).transpose(0, 2, 1, 3)
         .reshape(ng2, PART, W2GRP * d))


def _unpack_y(yt, d, nblk, tok_blk):
    """[nblk, 128, nd*tok_blk] f32 -> (d, nblk*tok_blk)."""
    nd = d // PART
    return (yt.reshape(nblk, PART, nd, tok_blk).transpose(2, 1, 0, 3)
              .reshape(d, nblk * tok_blk))


def kernel(**inputs) -> np.ndarray:
    xs = np.asarray(inputs["xs"], np.float32)
    bn = np.asarray(inputs["bottle_neck"], np.float32)
    Wlg = np.asarray(inputs["Wlg"], np.float32)
    blg = np.asarray(inputs["blg"], np.float32)
    Wg = np.asarray(inputs["Wg"], np.float32)
    W1 = np.asarray(inputs["W1"], np.float32)
    b1 = np.asarray(inputs["b1"], np.float32)
    W2 = np.asarray(inputs["W2"], np.float32)
    b2 = np.asarray(inputs["b2"], np.float32)
    k = int(np.asarray(inputs["top_k"]))

    Bx, Lx, d = xs.shape
    hdim = W1.shape[2]
    N = Bx * Lx
    nh = hdim // PART
    nd = d // PART
    x = xs.reshape(N, d)
    bnf = bn.reshape(N, d)

    sel_group, assign = _route(x, bnf, Wlg, blg, Wg, k)

    # Token sets per (group, expert) core.
    idxs, wgts = [], []
    for c in range(NCORES):
        g, e = divmod(c, E)
        sel, w = assign[g]
        mask = (sel_group == g)[:, None] & (sel == e)
        rows, cols = np.nonzero(mask)
        idxs.append(rows)
        wgts.append(w[rows, cols])

    cnt_max = max(len(i) for i in idxs)
    nfull = max(1, cnt_max // TOK_BLK)
    rem = max(0, cnt_max - nfull * TOK_BLK)
    rem_pad = -(-rem // 32) * 32 if rem else 0
    span0 = TOK_BLK + rem_pad

    nc = _build(nfull, rem, rem_pad, d, hdim)

    h_offsets = np.concatenate(([0], np.cumsum(W1CHUNKS)))[:-1]
    in_maps = []
    for c in range(NCORES):
        g, e = divmod(c, E)
        cnt = len(idxs[c])
        # token slot layout: block0 main [0:512], blocks 1.. [512:512*nfull],
        # rem tokens [512*nfull : 512*nfull+rem] (interleaved into block 0)
        X = np.zeros((nfull * TOK_BLK + rem_pad, d), np.float32)
        X[:cnt] = x[idxs[c]]
        x0 = np.concatenate([X[:TOK_BLK], X[nfull * TOK_BLK:]], axis=0)
        bfat = np.zeros((PART, BPAD), np.float32)
        bfat[:, :nh] = b1[g, e].reshape(nh, PART).T
        bfat[:, nh:nh + nd] = b2[g, e].reshape(nd, PART).T
        w1T = W1[g, e].T                          # (d, h)
        m = {
            "xt0": _pack_x(x0, d, 1, span0)[0],
            "w2t": _pack_w2(W2[g, e], d, hdim),
            "bt": bfat,
        }
        if nfull > 1:
            m["xt"] = _pack_x(X[TOK_BLK:nfull * TOK_BLK], d,
                              nfull - 1, TOK_BLK)
        for ci, w in enumerate(W1CHUNKS):
            m[f"w1c{ci}"] = _pack_w1_chunk(w1T, d, int(h_offsets[ci]), w)
        in_maps.append(m)

    res = bass_utils.run_bass_kernel_spmd(nc, in_maps, core_ids=list(range(NCORES)))

    out = np.zeros((N, d), np.float32)
    for c in range(NCORES):
        cnt = len(idxs[c])
        if cnt == 0:
            continue
        y_full = _unpack_y(res.results[c]["yt"], d, nfull, TOK_BLK)
        if rem:
            y_rem = _unpack_y(res.results[c]["yr"], d, 1, rem_pad)
            y_full = np.concatenate([y_full, y_rem], axis=1)
        yc = y_full[:, :cnt].T
        out[idxs[c]] += wgts[c][:, None] * yc
    return out.reshape(Bx, Lx, d).astype(np.float32)


# revision 21
# speedup vs baseline: 1.0277x; 1.0277x over previous
"""Group MoE layer (2 groups x 4 experts, top-1 group / top-2 expert routing)
on 8 Trainium2 NeuronCores via expert parallelism.

Strategy:
  - Host computes the (tiny) routing: language-gate argmax over groups,
    per-group expert top-k + softmax weights.
  - Tokens are dispatched by (group, expert) assignment: core c = g*4+e
    receives exactly the tokens routed to expert (g, e), padded to a common
    capacity C (SPMD: all cores run the same program).
  - Each core runs the dense FFN for its expert:
        Y^T = W2 @ relu(W1 @ X^T + b1) + b2      (tokens in the moving dim)
    with bf16 weights/activations and fp32 PSUM accumulation.
  - The capacity remainder above a multiple of 512 (e.g. C=1051 -> 2x512+27)
    does NOT get its own pass over the weights: a separate pass costs a full
    LDWEIGHTS sweep (512 weight tiles x ~56ns ~= 29us) regardless of token
    count. Instead the remainder rides block 0's weight stream: each weight
    tile issues a second matmul (N=rem) reusing the stationary operand,
    costing ~25ns extra per tile (~13us total instead of 29us).

HW behaviors this schedule is built around (all measured via ntff traces):
  - DMA transfers whose per-partition rows are < 8KB crawl (~34GB/s) while
    any engine is busy, so every tensor is packed with >= 8KB rows: the rem
    x-columns are interleaved into x block 0, and both biases live in one
    zero-padded [128, 2080] f32 tensor.
  - The scalar engine executes the activations AND its DMA triggers in one
    FIFO; a trigger blocks once the HWDGE ring saturates (~4+ queued), which
    deadlocks ACT -> PSUM-free -> matmul. So the scalar ring carries exactly
    2 triggers; the whole W1 -> W2 -> x stream serializes on the sync ring
    in consumption order.
  - The compile-time Tile scheduler prices DMA optimistically and coalesces
    semaphore thresholds, so mm2's accumulation runs hi high->low: its first
    matmul then depends on the LAST h1 tile and can't be hoisted (with a
    not-yet-landed W2 wait) into mm1's stream.
  - The PE's HAM clock gate holds 1.2GHz until ~3.4us of sustained activity:
    ~36 throwaway matmuls on a zeroed scratch tile warm it up during the
    initial loads, so the real stream starts at 2.4GHz.
"""

import numpy as np
import ml_dtypes

import concourse.bacc as bacc
import concourse.mybir as mybir
from concourse import tile
from concourse import bass_utils

B, L, D, H = 2, 2048, 1024, 4096
G, E = 2, 4
NCORES = G * E
PART = 128
TOK_BLK = 512
W2GRP = 4                       # h-tiles per merged W2 tile (8KB rows, 1MB)
BPAD = 2080                     # bias tensor f32 cols (8KB+ rows: no crawl)
# W1 chunk widths (columns of H per DMA); the first is smallest so the PE
# start only gates on 1MB of W1.
W1CHUNKS = (512, 512, 1024, 1024, 1024)
assert sum(W1CHUNKS) == H

_BF16 = ml_dtypes.bfloat16

_program_cache: dict[tuple, object] = {}


def _build(nfull: int, rem: int, rem_pad: int, d: int = D, h: int = H):
    """Per-core expert FFN program: nfull token blocks of 512 plus an
    optional remainder of `rem` tokens merged into block 0's weight stream
    (the rem columns are interleaved into block 0's x layout)."""
    key = (nfull, rem, rem_pad, d, h)
    if key in _program_cache:
        return _program_cache[key]

    nd = d // PART
    nh = h // PART
    ng2 = nh // W2GRP
    span0 = TOK_BLK + rem_pad           # block 0 carries the rem columns
    chunk_of = []
    for c, w in enumerate(W1CHUNKS):
        for off in range(w // PART):
            chunk_of.append((c, off))
    assert len(chunk_of) == nh

    bf16 = mybir.dt.bfloat16
    f32 = mybir.dt.float32

    nc = bacc.Bacc("TRN2", target_bir_lowering=False, debug=False,
                   num_devices=NCORES)

    xt0 = nc.dram_tensor("xt0", [PART, nd * span0], bf16,
                         kind="ExternalInput")
    if nfull > 1:
        xt = nc.dram_tensor("xt", [nfull - 1, PART, nd * TOK_BLK], bf16,
                            kind="ExternalInput")
    w1c = [nc.dram_tensor(f"w1c{c}", [PART, nd * w], bf16,
                          kind="ExternalInput")
           for c, w in enumerate(W1CHUNKS)]
    w2t = nc.dram_tensor("w2t", [ng2, PART, W2GRP * d], bf16,
                         kind="ExternalInput")
    bt = nc.dram_tensor("bt", [PART, BPAD], f32, kind="ExternalInput")
    yt = nc.dram_tensor("yt", [nfull, PART, nd * TOK_BLK], f32,
                        kind="ExternalOutput")
    if rem:
        yr = nc.dram_tensor("yr", [PART, nd * rem_pad], f32,
                            kind="ExternalOutput")

    with tile.TileContext(nc) as tc:
        with (
            tc.tile_pool(name="wpool", bufs=1) as wpool,
            tc.tile_pool(name="h1pool", bufs=nh) as h1pool,
            tc.tile_pool(name="ypool", bufs=1) as ypool,
            tc.tile_pool(name="ps1", bufs=2, space="PSUM") as ps1,
            tc.tile_pool(name="ps2", bufs=2, space="PSUM") as ps2,
        ):
            # PE warm-up (HAM clock gate, see module docstring)
            warm_x = wpool.tile([PART, TOK_BLK], bf16, tag="warm")
            nc.gpsimd.memset(warm_x[:, :], 0.0)
            warm_ps = ps1.tile([PART, TOK_BLK], f32, tag="m")
            for _ in range(36):
                nc.tensor.matmul(warm_ps[:, :], warm_x[:, :PART],
                                 warm_x[:, :], start=True, stop=True)

            w1_sb = [None] * len(W1CHUNKS)

            def load_w1(c, eng):
                t = wpool.tile([PART, nd * W1CHUNKS[c]], bf16, tag=f"w1_{c}")
                eng.dma_start(out=t[:, :], in_=w1c[c].ap()[:, :])
                w1_sb[c] = t

            # scalar ring: exactly ONE trigger (x block 0) so the ACT FIFO
            # behind it can never block on ring saturation
            x0_sb = wpool.tile([PART, nd * span0], bf16, tag="x_0")
            nc.scalar.dma_start(out=x0_sb[:, :], in_=xt0.ap()[:, :])

            # sync ring: biases first (needed by the first ACT), then W1 in
            # consumption order, W2, later x blocks
            x_sb = [None] * nfull
            x_sb[0] = x0_sb
            b_sb = wpool.tile([PART, BPAD], f32, tag="bt")
            nc.sync.dma_start(out=b_sb[:, :], in_=bt.ap()[:, :])
            for c in range(len(W1CHUNKS)):
                load_w1(c, nc.sync)
            w2_sb = [None] * ng2
            for gi in range(ng2):
                t = wpool.tile([PART, W2GRP * d], bf16, tag=f"w2_{gi}")
                nc.sync.dma_start(out=t[:, :], in_=w2t.ap()[gi])
                w2_sb[gi] = t
            for blk in range(1, nfull):
                t = wpool.tile([PART, nd * TOK_BLK], bf16, tag=f"x_{blk}")
                nc.sync.dma_start(out=t[:, :], in_=xt.ap()[blk - 1])
                x_sb[blk] = t

            # --- compute passes ------------------------------------------
            for p in range(nfull):
                merged = (p == 0 and rem > 0)
                span = span0 if p == 0 else TOK_BLK
                h1m_tiles = []
                h1r_tiles = []
                for hi in range(nh):
                    c, off = chunk_of[hi]
                    wch = W1CHUNKS[c]
                    psm = ps1.tile([PART, TOK_BLK], f32, tag="m")
                    if merged:
                        psr = ps1.tile([PART, TOK_BLK], f32, tag="r")
                    for di in range(nd):
                        w_ap = w1_sb[c][:, di * wch + off * PART:
                                        di * wch + (off + 1) * PART]
                        nc.tensor.matmul(
                            psm[:, :], w_ap,
                            x_sb[p][:, di * span:di * span + TOK_BLK],
                            start=(di == 0), stop=(di == nd - 1),
                        )
                        if merged:
                            # second matmul on the same stationary weights
                            nc.tensor.matmul(
                                psr[:, :rem], w_ap,
                                x_sb[0][:, di * span + TOK_BLK:
                                        di * span + TOK_BLK + rem],
                                start=(di == 0), stop=(di == nd - 1),
                            )
                    h1m = h1pool.tile([PART, TOK_BLK], bf16, tag="h1m")
                    nc.scalar.activation(h1m[:, :], psm[:, :],
                                         mybir.ActivationFunctionType.Relu,
                                         bias=b_sb[:, hi:hi + 1], scale=1.0)
                    h1m_tiles.append(h1m)
                    if merged:
                        h1r = h1pool.tile([PART, rem_pad], bf16, tag="h1r")
                        nc.scalar.activation(
                            h1r[:, :rem], psr[:, :rem],
                            mybir.ActivationFunctionType.Relu,
                            bias=b_sb[:, hi:hi + 1], scale=1.0)
                        h1r_tiles.append(h1r)

                y = ypool.tile([PART, nd * TOK_BLK], f32, tag="y")
                if merged:
                    y_r = ypool.tile([PART, nd * rem_pad], f32, tag="yr")
                for di in range(nd):
                    ps2m = ps2.tile([PART, TOK_BLK], f32, tag="m")
                    if merged:
                        ps2r = ps2.tile([PART, TOK_BLK], f32, tag="r")
                    # hi runs high->low: the chain's first matmul then needs
                    # the LAST h1 tile, so the compile-time scheduler cannot
                    # hoist mm2 matmuls (whose W2 may still be in flight)
                    # ahead of ready mm1 work in the in-order PE queue.
                    for hi in range(nh - 1, -1, -1):
                        gi, hj = divmod(hi, W2GRP)
                        w_ap = w2_sb[gi][:, hj * d + di * PART:
                                         hj * d + (di + 1) * PART]
                        nc.tensor.matmul(
                            ps2m[:, :], w_ap, h1m_tiles[hi][:, :],
                            start=(hi == nh - 1), stop=(hi == 0),
                        )
                        if merged:
                            nc.tensor.matmul(
                                ps2r[:, :rem], w_ap, h1r_tiles[hi][:, :rem],
                                start=(hi == nh - 1), stop=(hi == 0),
                            )
                    nc.vector.tensor_scalar_add(
                        y[:, di * TOK_BLK:(di + 1) * TOK_BLK], ps2m[:, :],
                        b_sb[:, nh + di:nh + di + 1])
                    # drain several d-tiles per DMA (8KB rows go at line
                    # rate), but keep the final drain a single d-tile so the
                    # post-last-matmul tail transfer is small
                    if di in (3, nd - 2, nd - 1):
                        lo = 0 if di == 3 else (4 if di == nd - 2 else nd - 1)
                        nc.sync.dma_start(
                            out=yt.ap()[p][:, lo * TOK_BLK:(di + 1) * TOK_BLK],
                            in_=y[:, lo * TOK_BLK:(di + 1) * TOK_BLK])
                    if merged:
                        nc.vector.tensor_scalar_add(
                            y_r[:, di * rem_pad:di * rem_pad + rem],
                            ps2r[:, :rem], b_sb[:, nh + di:nh + di + 1])
                if merged:
                    # one drain for the whole remainder block (mid-kernel,
                    # fully overlapped with the rest of the compute)
                    nc.sync.dma_start(out=yr.ap()[:, :], in_=y_r[:, :])

    nc.compile()
    _program_cache[key] = nc
    return nc


def _route(x, bn, Wlg, blg, Wg, k):
    """Numpy replica of the reference routing. Returns per-(g,e) assignment."""
    glog = bn @ Wlg.T + blg                       # (N, G)
    sel_group = np.argmax(glog, axis=1)           # (N,)
    assign = []
    for g in range(Wg.shape[0]):
        logits = x @ Wg[g].T                      # (N, E)
        order = np.argsort(-logits, axis=1, kind="stable")
        sel = order[:, :k]                        # (N, k)
        top = np.take_along_axis(logits, sel, axis=1).astype(np.float32)
        m = top.max(axis=1, keepdims=True)
        ex = np.exp(top - m)
        w = ex / ex.sum(axis=1, keepdims=True)    # (N, k)
        assign.append((sel, w))
    return sel_group, assign


def _pack_x(X, d, nblk, tok_blk):
    """(nblk*tok_blk, d) fp32 -> [nblk, 128, nd*tok_blk] bf16 merged tiles."""
    nd = d // PART
    xt = X.T.astype(_BF16)                        # (d, nblk*tok_blk)
    return np.ascontiguousarray(
        xt.reshape(nd, PART, nblk, tok_blk).transpose(2, 1, 0, 3)
          .reshape(nblk, PART, nd * tok_blk))


def _pack_w1_chunk(W1e_T, d, h0, w):
    """W1e.T slice (d, h0:h0+w) fp32 -> [128, nd*w] bf16."""
    nd = d // PART
    wsl = W1e_T[:, h0:h0 + w].astype(_BF16)       # (d, w)
    return np.ascontiguousarray(
        wsl.reshape(nd, PART, w).transpose(1, 0, 2).reshape(PART, nd * w))


def _pack_w2(W2e, d, h):
    ng2 = h // PART // W2GRP
    w = W2e.T.astype(_BF16)                       # (h, d)
    return np.ascontiguousarray(
        w.reshape(ng2, W2GRP, PART, d).transpose(0, 2, 1, 3)
         .reshape(ng2, PART, W2GRP * d))


def _unpack_y(yt, d, nblk, tok_blk):
    """[nblk, 128, nd*tok_blk] f32 -> (d, nblk*tok_blk)."""
    nd = d // PART
    return (yt.reshape(nblk, PART, nd, tok_blk).transpose(2, 1, 0, 3)
              .reshape(d, nblk * tok_blk))


def kernel(**inputs) -> np.ndarray:
    xs = np.asarray(inputs["xs"], np.float32)
    bn = np.asarray(inputs["bottle_neck"], np.float32)
    Wlg = np.asarray(inputs["Wlg"], np.float32)
    blg = np.asarray(inputs["blg"], np.float32)
    Wg = np.asarray(inputs["Wg"], np.float32)
    W1 = np.asarray(inputs["W1"], np.float32)
    b1 = np.asarray(inputs["b1"], np.float32)
    W2 = np.asarray(inputs["W2"], np.float32)
    b2 = np.asarray(inputs["b2"], np.float32)
    k = int(np.asarray(inputs["top_k"]))

    Bx, Lx, d = xs.shape
    hdim = W1.shape[2]
    N = Bx * Lx
    nh = hdim // PART
    nd = d // PART
    x = xs.reshape(N, d)
    bnf = bn.reshape(N, d)

    sel_group, assign = _route(x, bnf, Wlg, blg, Wg, k)

    # Token sets per (group, expert) core.
    idxs, wgts = [], []
    for c in range(NCORES):
        g, e = divmod(c, E)
        sel, w = assign[g]
        mask = (sel_group == g)[:, None] & (sel == e)
        rows, cols = np.nonzero(mask)
        idxs.append(rows)
        wgts.append(w[rows, cols])

    cnt_max = max(len(i) for i in idxs)
    nfull = max(1, cnt_max // TOK_BLK)
    rem = max(0, cnt_max - nfull * TOK_BLK)
    rem_pad = -(-rem // 32) * 32 if rem else 0
    span0 = TOK_BLK + rem_pad

    nc = _build(nfull, rem, rem_pad, d, hdim)

    h_offsets = np.concatenate(([0], np.cumsum(W1CHUNKS)))[:-1]
    in_maps = []
    for c in range(NCORES):
        g, e = divmod(c, E)
        cnt = len(idxs[c])
        # token slot layout: block0 main [0:512], blocks 1.. [512:512*nfull],
        # rem tokens [512*nfull : 512*nfull+rem] (interleaved into block 0)
        X = np.zeros((nfull * TOK_BLK + rem_pad, d), np.float32)
        X[:cnt] = x[idxs[c]]
        x0 = np.concatenate([X[:TOK_BLK], X[nfull * TOK_BLK:]], axis=0)
        bfat = np.zeros((PART, BPAD), np.float32)
        bfat[:, :nh] = b1[g, e].reshape(nh, PART).T
        bfat[:, nh:nh + nd] = b2[g, e].reshape(nd, PART).T
        w1T = W1[g, e].T                          # (d, h)
        m = {
            "xt0": _pack_x(x0, d, 1, span0)[0],
            "w2t": _pack_w2(W2[g, e], d, hdim),
            "bt": bfat,
        }
        if nfull > 1:
            m["xt"] = _pack_x(X[TOK_BLK:nfull * TOK_BLK], d,
                              nfull - 1, TOK_BLK)
        for ci, w in enumerate(W1CHUNKS):
            m[f"w1c{ci}"] = _pack_w1_chunk(w1T, d, int(h_offsets[ci]), w)
        in_maps.append(m)

    res = bass_utils.run_bass_kernel_spmd(nc, in_maps, core_ids=list(range(NCORES)))

    out = np.zeros((N, d), np.float32)
    for c in range(NCORES):
        cnt = len(idxs[c])
        if cnt == 0:
            continue
        y_full = _unpack_y(res.results[c]["yt"], d, nfull, TOK_BLK)
        if rem:
            y_rem = _unpack_y(res.results[c]["yr"], d, 1, rem_pad)
            y_full = np.concatenate([y_full, y_rem], axis=1)
        yc = y_full[:, :cnt].T
        out[idxs[c]] += wgts[c][:, None] * yc
    return out.reshape(Bx, Lx, d).astype(np.float32)


# revision 23
# speedup vs baseline: 1.0301x; 1.0024x over previous
"""Group MoE layer (2 groups x 4 experts, top-1 group / top-2 expert routing)
on 8 Trainium2 NeuronCores via expert parallelism.

Strategy:
  - Host computes the (tiny) routing: language-gate argmax over groups,
    per-group expert top-k + softmax weights.
  - Tokens are dispatched by (group, expert) assignment: core c = g*4+e
    receives exactly the tokens routed to expert (g, e), padded to a common
    capacity C (SPMD: all cores run the same program).
  - Each core runs the dense FFN for its expert:
        Y^T = W2 @ relu(W1 @ X^T + b1) + b2      (tokens in the moving dim)
    with bf16 weights/activations and fp32 PSUM accumulation.
  - The capacity remainder above a multiple of 512 (e.g. C=1051 -> 2x512+27)
    does NOT get its own pass over the weights: a separate pass costs a full
    LDWEIGHTS sweep (512 weight tiles x ~56ns ~= 29us) regardless of token
    count. Instead the remainder rides block 0's weight stream: each weight
    tile issues a second matmul (N=rem) reusing the stationary operand,
    costing ~25ns extra per tile (~13us total instead of 29us).

HW behaviors this schedule is built around (all measured via ntff traces):
  - DMA transfers whose per-partition rows are < 8KB crawl (~34GB/s) while
    any engine is busy, so every tensor is packed with >= 8KB rows: the rem
    x-columns are interleaved into x block 0, and both biases live in one
    zero-padded [128, 2080] f32 tensor.
  - The scalar engine executes the activations AND its DMA triggers in one
    FIFO; a trigger blocks once the HWDGE ring saturates (~4+ queued), which
    deadlocks ACT -> PSUM-free -> matmul. So the scalar ring carries exactly
    2 triggers; the whole W1 -> W2 -> x stream serializes on the sync ring
    in consumption order.
  - The compile-time Tile scheduler prices DMA optimistically and coalesces
    semaphore thresholds, so mm2's accumulation runs hi high->low: its first
    matmul then depends on the LAST h1 tile and can't be hoisted (with a
    not-yet-landed W2 wait) into mm1's stream.
  - The PE's HAM clock gate holds 1.2GHz until ~3.4us of sustained activity:
    ~36 throwaway matmuls on a zeroed scratch tile warm it up during the
    initial loads, so the real stream starts at 2.4GHz.
"""

import numpy as np
import ml_dtypes

import concourse.bacc as bacc
import concourse.mybir as mybir
from concourse import tile
from concourse import bass_utils

B, L, D, H = 2, 2048, 1024, 4096
G, E = 2, 4
NCORES = G * E
PART = 128
TOK_BLK = 512
W2GRP = 4                       # h-tiles per merged W2 tile (8KB rows, 1MB)
BPAD = 2080                     # bias tensor f32 cols (8KB+ rows: no crawl)
# W1 chunk widths (columns of H per DMA); the first is smallest so the PE
# start only gates on 1MB of W1.
W1CHUNKS = (512, 512, 1024, 1024, 1024)
assert sum(W1CHUNKS) == H

_BF16 = ml_dtypes.bfloat16

_program_cache: dict[tuple, object] = {}


def _build(nfull: int, rem: int, rem_pad: int, d: int = D, h: int = H):
    """Per-core expert FFN program: nfull token blocks of 512 plus an
    optional remainder of `rem` tokens merged into block 0's weight stream
    (the rem columns are interleaved into block 0's x layout)."""
    key = (nfull, rem, rem_pad, d, h)
    if key in _program_cache:
        return _program_cache[key]

    nd = d // PART
    nh = h // PART
    ng2 = nh // W2GRP
    span0 = TOK_BLK + rem_pad           # block 0 carries the rem columns
    chunk_of = []
    for c, w in enumerate(W1CHUNKS):
        for off in range(w // PART):
            chunk_of.append((c, off))
    assert len(chunk_of) == nh

    bf16 = mybir.dt.bfloat16
    f32 = mybir.dt.float32

    nc = bacc.Bacc("TRN2", target_bir_lowering=False, debug=False,
                   num_devices=NCORES)

    xt0 = nc.dram_tensor("xt0", [PART, nd * span0], bf16,
                         kind="ExternalInput")
    if nfull > 1:
        xt = nc.dram_tensor("xt", [nfull - 1, PART, nd * TOK_BLK], bf16,
                            kind="ExternalInput")
    w1c = [nc.dram_tensor(f"w1c{c}", [PART, nd * w], bf16,
                          kind="ExternalInput")
           for c, w in enumerate(W1CHUNKS)]
    w2t = nc.dram_tensor("w2t", [ng2, PART, W2GRP * d], bf16,
                         kind="ExternalInput")
    bt = nc.dram_tensor("bt", [PART, BPAD], f32, kind="ExternalInput")
    yt = nc.dram_tensor("yt", [nfull, PART, nd * TOK_BLK], f32,
                        kind="ExternalOutput")
    if rem:
        yr = nc.dram_tensor("yr", [PART, nd * rem_pad], f32,
                            kind="ExternalOutput")

    with tile.TileContext(nc) as tc:
        with (
            tc.tile_pool(name="wpool", bufs=1) as wpool,
            tc.tile_pool(name="h1pool", bufs=nh) as h1pool,
            tc.tile_pool(name="ypool", bufs=1) as ypool,
            tc.tile_pool(name="ps1", bufs=2, space="PSUM") as ps1,
            tc.tile_pool(name="ps2", bufs=2, space="PSUM") as ps2,
        ):
            # PE warm-up (HAM clock gate, see module docstring)
            warm_x = wpool.tile([PART, TOK_BLK], bf16, tag="warm")
            nc.gpsimd.memset(warm_x[:, :], 0.0)
            warm_ps = ps1.tile([PART, TOK_BLK], f32, tag="m")
            for _ in range(24):
                nc.tensor.matmul(warm_ps[:, :], warm_x[:, :PART],
                                 warm_x[:, :], start=True, stop=True)

            w1_sb = [None] * len(W1CHUNKS)

            def load_w1(c, eng):
                t = wpool.tile([PART, nd * W1CHUNKS[c]], bf16, tag=f"w1_{c}")
                eng.dma_start(out=t[:, :], in_=w1c[c].ap()[:, :])
                w1_sb[c] = t

            # scalar ring: exactly ONE trigger (x block 0) so the ACT FIFO
            # behind it can never block on ring saturation
            x0_sb = wpool.tile([PART, nd * span0], bf16, tag="x_0")
            nc.scalar.dma_start(out=x0_sb[:, :], in_=xt0.ap()[:, :])

            # sync ring: W1 chunk 0 (gates the first matmul), biases (needed
            # by the first ACT ~2us later), the rest of W1 in consumption
            # order, W2, later x blocks
            x_sb = [None] * nfull
            x_sb[0] = x0_sb
            load_w1(0, nc.sync)
            b_sb = wpool.tile([PART, BPAD], f32, tag="bt")
            nc.sync.dma_start(out=b_sb[:, :], in_=bt.ap()[:, :])
            for c in range(1, len(W1CHUNKS)):
                load_w1(c, nc.sync)
            w2_sb = [None] * ng2
            for gi in range(ng2):
                t = wpool.tile([PART, W2GRP * d], bf16, tag=f"w2_{gi}")
                nc.sync.dma_start(out=t[:, :], in_=w2t.ap()[gi])
                w2_sb[gi] = t
            for blk in range(1, nfull):
                t = wpool.tile([PART, nd * TOK_BLK], bf16, tag=f"x_{blk}")
                nc.sync.dma_start(out=t[:, :], in_=xt.ap()[blk - 1])
                x_sb[blk] = t

            # --- compute passes ------------------------------------------
            for p in range(nfull):
                merged = (p == 0 and rem > 0)
                span = span0 if p == 0 else TOK_BLK
                h1m_tiles = []
                h1r_tiles = []
                for hi in range(nh):
                    c, off = chunk_of[hi]
                    wch = W1CHUNKS[c]
                    psm = ps1.tile([PART, TOK_BLK], f32, tag="m")
                    if merged:
                        psr = ps1.tile([PART, TOK_BLK], f32, tag="r")
                    for di in range(nd):
                        w_ap = w1_sb[c][:, di * wch + off * PART:
                                        di * wch + (off + 1) * PART]
                        nc.tensor.matmul(
                            psm[:, :], w_ap,
                            x_sb[p][:, di * span:di * span + TOK_BLK],
                            start=(di == 0), stop=(di == nd - 1),
                        )
                        if merged:
                            # second matmul on the same stationary weights
                            nc.tensor.matmul(
                                psr[:, :rem], w_ap,
                                x_sb[0][:, di * span + TOK_BLK:
                                        di * span + TOK_BLK + rem],
                                start=(di == 0), stop=(di == nd - 1),
                            )
                    h1m = h1pool.tile([PART, TOK_BLK], bf16, tag="h1m")
                    nc.scalar.activation(h1m[:, :], psm[:, :],
                                         mybir.ActivationFunctionType.Relu,
                                         bias=b_sb[:, hi:hi + 1], scale=1.0)
                    h1m_tiles.append(h1m)
                    if merged:
                        h1r = h1pool.tile([PART, rem_pad], bf16, tag="h1r")
                        nc.scalar.activation(
                            h1r[:, :rem], psr[:, :rem],
                            mybir.ActivationFunctionType.Relu,
                            bias=b_sb[:, hi:hi + 1], scale=1.0)
                        h1r_tiles.append(h1r)

                y = ypool.tile([PART, nd * TOK_BLK], f32, tag="y")
                if merged:
                    y_r = ypool.tile([PART, nd * rem_pad], f32, tag="yr")
                for di in range(nd):
                    ps2m = ps2.tile([PART, TOK_BLK], f32, tag="m")
                    if merged:
                        ps2r = ps2.tile([PART, TOK_BLK], f32, tag="r")
                    # hi runs high->low: the chain's first matmul then needs
                    # the LAST h1 tile, so the compile-time scheduler cannot
                    # hoist mm2 matmuls (whose W2 may still be in flight)
                    # ahead of ready mm1 work in the in-order PE queue.
                    for hi in range(nh - 1, -1, -1):
                        gi, hj = divmod(hi, W2GRP)
                        w_ap = w2_sb[gi][:, hj * d + di * PART:
                                         hj * d + (di + 1) * PART]
                        nc.tensor.matmul(
                            ps2m[:, :], w_ap, h1m_tiles[hi][:, :],
                            start=(hi == nh - 1), stop=(hi == 0),
                        )
                        if merged:
                            nc.tensor.matmul(
                                ps2r[:, :rem], w_ap, h1r_tiles[hi][:, :rem],
                                start=(hi == nh - 1), stop=(hi == 0),
                            )
                    nc.vector.tensor_scalar_add(
                        y[:, di * TOK_BLK:(di + 1) * TOK_BLK], ps2m[:, :],
                        b_sb[:, nh + di:nh + di + 1])
                    # drain several d-tiles per DMA (8KB rows go at line
                    # rate), but keep the final drain a single d-tile so the
                    # post-last-matmul tail transfer is small
                    if di in (3, nd - 2, nd - 1):
                        lo = 0 if di == 3 else (4 if di == nd - 2 else nd - 1)
                        nc.sync.dma_start(
                            out=yt.ap()[p][:, lo * TOK_BLK:(di + 1) * TOK_BLK],
                            in_=y[:, lo * TOK_BLK:(di + 1) * TOK_BLK])
                    if merged:
                        nc.vector.tensor_scalar_add(
                            y_r[:, di * rem_pad:di * rem_pad + rem],
                            ps2r[:, :rem], b_sb[:, nh + di:nh + di + 1])
                if merged:
                    # one drain for the whole remainder block (mid-kernel,
                    # fully overlapped with the rest of the compute)
                    nc.sync.dma_start(out=yr.ap()[:, :], in_=y_r[:, :])

    nc.compile()
    _program_cache[key] = nc
    return nc


def _route(x, bn, Wlg, blg, Wg, k):
    """Numpy replica of the reference routing. Returns per-(g,e) assignment."""
    glog = bn @ Wlg.T + blg                       # (N, G)
    sel_group = np.argmax(glog, axis=1)           # (N,)
    assign = []
    for g in range(Wg.shape[0]):
        logits = x @ Wg[g].T                      # (N, E)
        order = np.argsort(-logits, axis=1, kind="stable")
        sel = order[:, :k]                        # (N, k)
        top = np.take_along_axis(logits, sel, axis=1).astype(np.float32)
        m = top.max(axis=1, keepdims=True)
        ex = np.exp(top - m)
        w = ex / ex.sum(axis=1, keepdims=True)    # (N, k)
        assign.append((sel, w))
    return sel_group, assign


def _pack_x(X, d, nblk, tok_blk):
    """(nblk*tok_blk, d) fp32 -> [nblk, 128, nd*tok_blk] bf16 merged tiles."""
    nd = d // PART
    xt = X.T.astype(_BF16)                        # (d, nblk*tok_blk)
    return np.ascontiguousarray(
        xt.reshape(nd, PART, nblk, tok_blk).transpose(2, 1, 0, 3)
          .reshape(nblk, PART, nd * tok_blk))


def _pack_w1_chunk(W1e_T, d, h0, w):
    """W1e.T slice (d, h0:h0+w) fp32 -> [128, nd*w] bf16."""
    nd = d // PART
    wsl = W1e_T[:, h0:h0 + w].astype(_BF16)       # (d, w)
    return np.ascontiguousarray(
        wsl.reshape(nd, PART, w).transpose(1, 0, 2).reshape(PART, nd * w))


def _pack_w2(W2e, d, h):
    ng2 = h // PART // W2GRP
    w = W2e.T.astype(_BF16)                       # (h, d)
    return np.ascontiguousarray(
        w.reshape(ng2, W2GRP, PART, d).transpose(0, 2, 1, 3)
         .reshape(ng2, PART, W2GRP * d))


def _unpack_y(yt, d, nblk, tok_blk):
    """[nblk, 128, nd*tok_blk] f32 -> (d, nblk*tok_blk)."""
    nd = d // PART
    return (yt.reshape(nblk, PART, nd, tok_blk).transpose(2, 1, 0, 3)
              .reshape(d, nblk * tok_blk))


def kernel(**inputs) -> np.ndarray:
    xs = np.asarray(inputs["xs"], np.float32)
    bn = np.asarray(inputs["bottle_neck"], np.float32)
    Wlg = np.asarray(inputs["Wlg"], np.float32)
    blg = np.asarray(inputs["blg"], np.float32)
    Wg = np.asarray(inputs["Wg"], np.float32)
    W1 = np.asarray(inputs["W1"], np.float32)
    b1 = np.asarray(inputs["b1"], np.float32)
    W2 = np.asarray(inputs["W2"], np.float32)
    b2 = np.asarray(inputs["b2"], np.float32)
    k = int(np.asarray(inputs["top_k"]))

    Bx, Lx, d = xs.shape
    hdim = W1.shape[2]
    N = Bx * Lx
    nh = hdim // PART
    nd = d // PART
    x = xs.reshape(N, d)
    bnf = bn.reshape(N, d)

    sel_group, assign = _route(x, bnf, Wlg, blg, Wg, k)

    # Token sets per (group, expert) core.
    idxs, wgts = [], []
    for c in range(NCORES):
        g, e = divmod(c, E)
        sel, w = assign[g]
        mask = (sel_group == g)[:, None] & (sel == e)
        rows, cols = np.nonzero(mask)
        idxs.append(rows)
        wgts.append(w[rows, cols])

    cnt_max = max(len(i) for i in idxs)
    nfull = max(1, cnt_max // TOK_BLK)
    rem = max(0, cnt_max - nfull * TOK_BLK)
    rem_pad = -(-rem // 32) * 32 if rem else 0
    span0 = TOK_BLK + rem_pad

    nc = _build(nfull, rem, rem_pad, d, hdim)

    h_offsets = np.concatenate(([0], np.cumsum(W1CHUNKS)))[:-1]
    in_maps = []
    for c in range(NCORES):
        g, e = divmod(c, E)
        cnt = len(idxs[c])
        # token slot layout: block0 main [0:512], blocks 1.. [512:512*nfull],
        # rem tokens [512*nfull : 512*nfull+rem] (interleaved into block 0)
        X = np.zeros((nfull * TOK_BLK + rem_pad, d), np.float32)
        X[:cnt] = x[idxs[c]]
        x0 = np.concatenate([X[:TOK_BLK], X[nfull * TOK_BLK:]], axis=0)
        bfat = np.zeros((PART, BPAD), np.float32)
        bfat[:, :nh] = b1[g, e].reshape(nh, PART).T
        bfat[:, nh:nh + nd] = b2[g, e].reshape(nd, PART).T
        w1T = W1[g, e].T                          # (d, h)
        m = {
            "xt0": _pack_x(x0, d, 1, span0)[0],
            "w2t": _pack_w2(W2[g, e], d, hdim),
            "bt": bfat,
        }
        if nfull > 1:
            m["xt"] = _pack_x(X[TOK_BLK:nfull * TOK_BLK], d,
                              nfull - 1, TOK_BLK)
        for ci, w in enumerate(W1CHUNKS):
            m[f"w1c{ci}"] = _pack_w1_chunk(w1T, d, int(h_offsets[ci]), w)
        in_maps.append(m)

    res = bass_utils.run_bass_kernel_spmd(nc, in_maps, core_ids=list(range(NCORES)))

    out = np.zeros((N, d), np.float32)
    for c in range(NCORES):
        cnt = len(idxs[c])
        if cnt == 0:
            continue
        y_full = _unpack_y(res.results[c]["yt"], d, nfull, TOK_BLK)
        if rem:
            y_rem = _unpack_y(res.results[c]["yr"], d, 1, rem_pad)
            y_full = np.concatenate([y_full, y_rem], axis=1)
        yc = y_full[:, :cnt].T
        out[idxs[c]] += wgts[c][:, None] * yc
    return out.reshape(Bx, Lx, d).astype(np.float32)


# revision 24
# speedup vs baseline: 1.0343x; 1.0041x over previous
"""Group MoE layer (2 groups x 4 experts, top-1 group / top-2 expert routing)
on 8 Trainium2 NeuronCores via expert parallelism.

Strategy:
  - Host computes the (tiny) routing: language-gate argmax over groups,
    per-group expert top-k + softmax weights.
  - Tokens are dispatched by (group, expert) assignment: core c = g*4+e
    receives exactly the tokens routed to expert (g, e), padded to a common
    capacity C (SPMD: all cores run the same program).
  - Each core runs the dense FFN for its expert:
        Y^T = W2 @ relu(W1 @ X^T + b1) + b2      (tokens in the moving dim)
    with bf16 weights/activations and fp32 PSUM accumulation.
  - The capacity remainder above a multiple of 512 (e.g. C=1051 -> 2x512+27)
    does NOT get its own pass over the weights: a separate pass costs a full
    LDWEIGHTS sweep (512 weight tiles x ~56ns ~= 29us) regardless of token
    count. Instead the remainder rides block 0's weight stream: each weight
    tile issues a second matmul (N=rem) reusing the stationary operand,
    costing ~25ns extra per tile (~13us total instead of 29us).

HW behaviors this schedule is built around (all measured via ntff traces):
  - DMA transfers whose per-partition rows are < 8KB crawl (~34GB/s) while
    any engine is busy, so every tensor is packed with >= 8KB rows: the rem
    x-columns are interleaved into x block 0, and both biases live in one
    zero-padded [128, 2080] f32 tensor.
  - The scalar engine executes the activations AND its DMA triggers in one
    FIFO; a trigger blocks once the HWDGE ring saturates (~4+ queued), which
    deadlocks ACT -> PSUM-free -> matmul. So the scalar ring carries exactly
    2 triggers; the whole W1 -> W2 -> x stream serializes on the sync ring
    in consumption order.
  - The compile-time Tile scheduler prices DMA optimistically and coalesces
    semaphore thresholds, so mm2's accumulation runs hi high->low: its first
    matmul then depends on the LAST h1 tile and can't be hoisted (with a
    not-yet-landed W2 wait) into mm1's stream.
  - The PE's HAM clock gate holds 1.2GHz until ~3.4us of sustained activity:
    ~36 throwaway matmuls on a zeroed scratch tile warm it up during the
    initial loads, so the real stream starts at 2.4GHz.
"""

import numpy as np
import ml_dtypes

import concourse.bacc as bacc
import concourse.mybir as mybir
from concourse import tile
from concourse import bass_utils

B, L, D, H = 2, 2048, 1024, 4096
G, E = 2, 4
NCORES = G * E
PART = 128
TOK_BLK = 512
W2GRP = 4                       # h-tiles per merged W2 tile (8KB rows, 1MB)
BPAD = 2080                     # bias tensor f32 cols (8KB+ rows: no crawl)
# W1 chunk widths (columns of H per DMA); the first is smallest so the PE
# start only gates on 1MB of W1.
W1CHUNKS = (512, 512, 1024, 1024, 1024)
assert sum(W1CHUNKS) == H

_BF16 = ml_dtypes.bfloat16

_program_cache: dict[tuple, object] = {}


def _build(nfull: int, rem: int, rem_pad: int, d: int = D, h: int = H):
    """Per-core expert FFN program: nfull token blocks of 512 plus an
    optional remainder of `rem` tokens merged into block 0's weight stream
    (the rem columns are interleaved into block 0's x layout)."""
    key = (nfull, rem, rem_pad, d, h)
    if key in _program_cache:
        return _program_cache[key]

    nd = d // PART
    nh = h // PART
    ng2 = nh // W2GRP
    span0 = TOK_BLK + rem_pad           # block 0 carries the rem columns
    chunk_of = []
    for c, w in enumerate(W1CHUNKS):
        for off in range(w // PART):
            chunk_of.append((c, off))
    assert len(chunk_of) == nh

    bf16 = mybir.dt.bfloat16
    f32 = mybir.dt.float32

    nc = bacc.Bacc("TRN2", target_bir_lowering=False, debug=False,
                   num_devices=NCORES)

    xt0 = nc.dram_tensor("xt0", [PART, nd * span0], bf16,
                         kind="ExternalInput")
    if nfull > 1:
        xt = nc.dram_tensor("xt", [nfull - 1, PART, nd * TOK_BLK], bf16,
                            kind="ExternalInput")
    w1c = [nc.dram_tensor(f"w1c{c}", [PART, nd * w], bf16,
                          kind="ExternalInput")
           for c, w in enumerate(W1CHUNKS)]
    w2t = nc.dram_tensor("w2t", [ng2, PART, W2GRP * d], bf16,
                         kind="ExternalInput")
    bt = nc.dram_tensor("bt", [PART, BPAD], f32, kind="ExternalInput")
    yt = nc.dram_tensor("yt", [nfull, PART, nd * TOK_BLK], f32,
                        kind="ExternalOutput")
    if rem:
        yr = nc.dram_tensor("yr", [PART, nd * rem_pad], f32,
                            kind="ExternalOutput")

    with tile.TileContext(nc) as tc:
        with (
            tc.tile_pool(name="wpool", bufs=1) as wpool,
            tc.tile_pool(name="h1pool", bufs=nh) as h1pool,
            tc.tile_pool(name="ypool", bufs=1) as ypool,
            tc.tile_pool(name="ps1", bufs=2, space="PSUM") as ps1,
            tc.tile_pool(name="ps2", bufs=2, space="PSUM") as ps2,
        ):
            # PE warm-up (HAM clock gate, see module docstring)
            warm_x = wpool.tile([PART, TOK_BLK], bf16, tag="warm")
            nc.gpsimd.memset(warm_x[:, :], 0.0)
            warm_ps = ps1.tile([PART, TOK_BLK], f32, tag="m")
            for _ in range(33):
                nc.tensor.matmul(warm_ps[:, :], warm_x[:, :PART],
                                 warm_x[:, :], start=True, stop=True)

            w1_sb = [None] * len(W1CHUNKS)

            def load_w1(c, eng):
                t = wpool.tile([PART, nd * W1CHUNKS[c]], bf16, tag=f"w1_{c}")
                eng.dma_start(out=t[:, :], in_=w1c[c].ap()[:, :])
                w1_sb[c] = t

            # scalar ring: exactly ONE trigger (x block 0) so the ACT FIFO
            # behind it can never block on ring saturation
            x0_sb = wpool.tile([PART, nd * span0], bf16, tag="x_0")
            nc.scalar.dma_start(out=x0_sb[:, :], in_=xt0.ap()[:, :])

            # sync ring: W1 chunk 0 (gates the first matmul), biases (needed
            # by the first ACT ~2us later), the rest of W1 in consumption
            # order, W2, later x blocks
            x_sb = [None] * nfull
            x_sb[0] = x0_sb
            load_w1(0, nc.sync)
            b_sb = wpool.tile([PART, BPAD], f32, tag="bt")
            nc.sync.dma_start(out=b_sb[:, :], in_=bt.ap()[:, :])
            for c in range(1, len(W1CHUNKS)):
                load_w1(c, nc.sync)
            w2_sb = [None] * ng2
            for gi in range(ng2):
                t = wpool.tile([PART, W2GRP * d], bf16, tag=f"w2_{gi}")
                nc.sync.dma_start(out=t[:, :], in_=w2t.ap()[gi])
                w2_sb[gi] = t
            for blk in range(1, nfull):
                t = wpool.tile([PART, nd * TOK_BLK], bf16, tag=f"x_{blk}")
                nc.sync.dma_start(out=t[:, :], in_=xt.ap()[blk - 1])
                x_sb[blk] = t

            # --- compute passes ------------------------------------------
            for p in range(nfull):
                merged = (p == 0 and rem > 0)
                span = span0 if p == 0 else TOK_BLK
                h1m_tiles = []
                h1r_tiles = []
                for hi in range(nh):
                    c, off = chunk_of[hi]
                    wch = W1CHUNKS[c]
                    psm = ps1.tile([PART, TOK_BLK], f32, tag="m")
                    if merged:
                        psr = ps1.tile([PART, TOK_BLK], f32, tag="r")
                    for di in range(nd):
                        w_ap = w1_sb[c][:, di * wch + off * PART:
                                        di * wch + (off + 1) * PART]
                        nc.tensor.matmul(
                            psm[:, :], w_ap,
                            x_sb[p][:, di * span:di * span + TOK_BLK],
                            start=(di == 0), stop=(di == nd - 1),
                        )
                        if merged:
                            # second matmul on the same stationary weights
                            nc.tensor.matmul(
                                psr[:, :rem], w_ap,
                                x_sb[0][:, di * span + TOK_BLK:
                                        di * span + TOK_BLK + rem],
                                start=(di == 0), stop=(di == nd - 1),
                            )
                    h1m = h1pool.tile([PART, TOK_BLK], bf16, tag="h1m")
                    nc.scalar.activation(h1m[:, :], psm[:, :],
                                         mybir.ActivationFunctionType.Relu,
                                         bias=b_sb[:, hi:hi + 1], scale=1.0)
                    h1m_tiles.append(h1m)
                    if merged:
                        h1r = h1pool.tile([PART, rem_pad], bf16, tag="h1r")
                        nc.scalar.activation(
                            h1r[:, :rem], psr[:, :rem],
                            mybir.ActivationFunctionType.Relu,
                            bias=b_sb[:, hi:hi + 1], scale=1.0)
                        h1r_tiles.append(h1r)

                y = ypool.tile([PART, nd * TOK_BLK], f32, tag="y")
                if merged:
                    y_r = ypool.tile([PART, nd * rem_pad], f32, tag="yr")
                for di in range(nd):
                    ps2m = ps2.tile([PART, TOK_BLK], f32, tag="m")
                    if merged:
                        ps2r = ps2.tile([PART, TOK_BLK], f32, tag="r")
                    # hi runs high->low: the chain's first matmul then needs
                    # the LAST h1 tile, so the compile-time scheduler cannot
                    # hoist mm2 matmuls (whose W2 may still be in flight)
                    # ahead of ready mm1 work in the in-order PE queue.
                    for hi in range(nh - 1, -1, -1):
                        gi, hj = divmod(hi, W2GRP)
                        w_ap = w2_sb[gi][:, hj * d + di * PART:
                                         hj * d + (di + 1) * PART]
                        nc.tensor.matmul(
                            ps2m[:, :], w_ap, h1m_tiles[hi][:, :],
                            start=(hi == nh - 1), stop=(hi == 0),
                        )
                        if merged:
                            nc.tensor.matmul(
                                ps2r[:, :rem], w_ap, h1r_tiles[hi][:, :rem],
                                start=(hi == nh - 1), stop=(hi == 0),
                            )
                    nc.vector.tensor_scalar_add(
                        y[:, di * TOK_BLK:(di + 1) * TOK_BLK], ps2m[:, :],
                        b_sb[:, nh + di:nh + di + 1])
                    # drain several d-tiles per DMA (8KB rows go at line
                    # rate), but keep the final drain a single d-tile so the
                    # post-last-matmul tail transfer is small
                    if di in (3, nd - 2, nd - 1):
                        lo = 0 if di == 3 else (4 if di == nd - 2 else nd - 1)
                        nc.sync.dma_start(
                            out=yt.ap()[p][:, lo * TOK_BLK:(di + 1) * TOK_BLK],
                            in_=y[:, lo * TOK_BLK:(di + 1) * TOK_BLK])
                    if merged:
                        nc.vector.tensor_scalar_add(
                            y_r[:, di * rem_pad:di * rem_pad + rem],
                            ps2r[:, :rem], b_sb[:, nh + di:nh + di + 1])
                if merged:
                    # one drain for the whole remainder block (mid-kernel,
                    # fully overlapped with the rest of the compute)
                    nc.sync.dma_start(out=yr.ap()[:, :], in_=y_r[:, :])

    nc.compile()
    _program_cache[key] = nc
    return nc


def _route(x, bn, Wlg, blg, Wg, k):
    """Numpy replica of the reference routing. Returns per-(g,e) assignment."""
    glog = bn @ Wlg.T + blg                       # (N, G)
    sel_group = np.argmax(glog, axis=1)           # (N,)
    assign = []
    for g in range(Wg.shape[0]):
        logits = x @ Wg[g].T                      # (N, E)
        order = np.argsort(-logits, axis=1, kind="stable")
        sel = order[:, :k]                        # (N, k)
        top = np.take_along_axis(logits, sel, axis=1).astype(np.float32)
        m = top.max(axis=1, keepdims=True)
        ex = np.exp(top - m)
        w = ex / ex.sum(axis=1, keepdims=True)    # (N, k)
        assign.append((sel, w))
    return sel_group, assign


def _pack_x(X, d, nblk, tok_blk):
    """(nblk*tok_blk, d) fp32 -> [nblk, 128, nd*tok_blk] bf16 merged tiles."""
    nd = d // PART
    xt = X.T.astype(_BF16)                        # (d, nblk*tok_blk)
    return np.ascontiguousarray(
        xt.reshape(nd, PART, nblk, tok_blk).transpose(2, 1, 0, 3)
          .reshape(nblk, PART, nd * tok_blk))


def _pack_w1_chunk(W1e_T, d, h0, w):
    """W1e.T slice (d, h0:h0+w) fp32 -> [128, nd*w] bf16."""
    nd = d // PART
    wsl = W1e_T[:, h0:h0 + w].astype(_BF16)       # (d, w)
    return np.ascontiguousarray(
        wsl.reshape(nd, PART, w).transpose(1, 0, 2).reshape(PART, nd * w))


def _pack_w2(W2e, d, h):
    ng2 = h // PART // W2GRP
    w = W2e.T.astype(_BF16)                       # (h, d)
    return np.ascontiguousarray(
        w.reshape(ng2, W2GRP, PART, d).transpose(0, 2, 1, 3)
         .reshape(ng2, PART, W2GRP * d))


def _unpack_y(yt, d, nblk, tok_blk):
    """[nblk, 128, nd*tok_blk] f32 -> (d, nblk*tok_blk)."""
    nd = d // PART
    return (yt.reshape(nblk, PART, nd, tok_blk).transpose(2, 1, 0, 3)
              .reshape(d, nblk * tok_blk))


def kernel(**inputs) -> np.ndarray:
    xs = np.asarray(inputs["xs"], np.float32)
    bn = np.asarray(inputs["bottle_neck"], np.float32)
    Wlg = np.asarray(inputs["Wlg"], np.float32)
    blg = np.asarray(inputs["blg"], np.float32)
    Wg = np.asarray(inputs["Wg"], np.float32)
    W1 = np.asarray(inputs["W1"], np.float32)
    b1 = np.asarray(inputs["b1"], np.float32)
    W2 = np.asarray(inputs["W2"], np.float32)
    b2 = np.asarray(inputs["b2"], np.float32)
    k = int(np.asarray(inputs["top_k"]))

    Bx, Lx, d = xs.shape
    hdim = W1.shape[2]
    N = Bx * Lx
    nh = hdim // PART
    nd = d // PART
    x = xs.reshape(N, d)
    bnf = bn.reshape(N, d)

    sel_group, assign = _route(x, bnf, Wlg, blg, Wg, k)

    # Token sets per (group, expert) core.
    idxs, wgts = [], []
    for c in range(NCORES):
        g, e = divmod(c, E)
        sel, w = assign[g]
        mask = (sel_group == g)[:, None] & (sel == e)
        rows, cols = np.nonzero(mask)
        idxs.append(rows)
        wgts.append(w[rows, cols])

    cnt_max = max(len(i) for i in idxs)
    nfull = max(1, cnt_max // TOK_BLK)
    rem = max(0, cnt_max - nfull * TOK_BLK)
    rem_pad = -(-rem // 32) * 32 if rem else 0
    span0 = TOK_BLK + rem_pad

    nc = _build(nfull, rem, rem_pad, d, hdim)

    h_offsets = np.concatenate(([0], np.cumsum(W1CHUNKS)))[:-1]
    in_maps = []
    for c in range(NCORES):
        g, e = divmod(c, E)
        cnt = len(idxs[c])
        # token slot layout: block0 main [0:512], blocks 1.. [512:512*nfull],
        # rem tokens [512*nfull : 512*nfull+rem] (interleaved into block 0)
        X = np.zeros((nfull * TOK_BLK + rem_pad, d), np.float32)
        X[:cnt] = x[idxs[c]]
        x0 = np.concatenate([X[:TOK_BLK], X[nfull * TOK_BLK:]], axis=0)
        bfat = np.zeros((PART, BPAD), np.float32)
        bfat[:, :nh] = b1[g, e].reshape(nh, PART).T
        bfat[:, nh:nh + nd] = b2[g, e].reshape(nd, PART).T
        w1T = W1[g, e].T                          # (d, h)
        m = {
            "xt0": _pack_x(x0, d, 1, span0)[0],
            "w2t": _pack_w2(W2[g, e], d, hdim),
            "bt": bfat,
        }
        if nfull > 1:
            m["xt"] = _pack_x(X[TOK_BLK:nfull * TOK_BLK], d,
                              nfull - 1, TOK_BLK)
        for ci, w in enumerate(W1CHUNKS):
            m[f"w1c{ci}"] = _pack_w1_chunk(w1T, d, int(h_offsets[ci]), w)
        in_maps.append(m)

    res = bass_utils.run_bass_kernel_spmd(nc, in_maps, core_ids=list(range(NCORES)))

    out = np.zeros((N, d), np.float32)
    for c in range(NCORES):
        cnt = len(idxs[c])
        if cnt == 0:
            continue
        y_full = _unpack_y(res.results[c]["yt"], d, nfull, TOK_BLK)
        if rem:
            y_rem = _unpack_y(res.results[c]["yr"], d, 1, rem_pad)
            y_full = np.concatenate([y_full, y_rem], axis=1)
        yc = y_full[:, :cnt].T
        out[idxs[c]] += wgts[c][:, None] * yc
    return out.reshape(Bx, Lx, d).astype(np.float32)
